# revision 1
# baseline (speedup 1.0000x reference)
"""Trainium2 Bass kernel for nn_MaskedAttention (B=2, N=2048, C=1024, H=16).

Sharding: batch x head-group over 8 cores (core c -> batch c//4, heads
4*(c%4)..4*(c%4)+3).  The reference's "faithful" head-scrambled reshape
means each head's output occupies a contiguous 128-row block of the
pre-projection matrix, so the output projection is row-parallel across
heads and needs no cross-core reduction.

Per-core pipeline (all matmuls fp32r / bf16 at 1 cycle/row):
  1. QKV projection: q,k stored transposed [d, n] with head pairs stacked
     on partitions (enables row-tiled K=64 score matmuls); v stored
     [j, d] per head augmented with a ones column (denominator trick).
  2. Scores transposed sT[j, i] = kT^T qT per 128x512 tile, causal tiles
     only; exp via ScalarE (scale/bias folded in, constant -20 bias for
     overflow safety); causal mask via memset + triangular multiply on
     diagonal tiles.
  3. out^T = [V | 1]^T @ expT accumulated over j chunks -> row 64 is the
     softmax denominator for free. Normalize with reciprocal + partition
     broadcast.
  4. Head-scramble staging (strided copies) + output projection + bias.
"""

import numpy as np

import concourse.bass as bass
import concourse.mybir as mybir
from concourse import tile
from concourse import library_config
from concourse.bass_utils import run_bass_kernel_spmd

B, N, C, H = 2, 2048, 1024, 16
D = C // H                 # 64
SCALE = D ** -0.5
EBIAS = -20.0
P = 128
NB = N // 512              # 4 i/n blocks
NJT = N // P               # 16 j tiles
F32 = mybir.dt.float32
F32R = mybir.dt.float32r
BF16 = mybir.dt.bfloat16
AF = mybir.ActivationFunctionType


def _emit(nc: bass.Bass, d: dict, repeats: int = 1):
    from contextlib import ExitStack

    with tile.TileContext(nc) as tc, ExitStack() as ctx:
        const = ctx.enter_context(tc.tile_pool(name="const", bufs=1))
        wqk = const.tile([P, 8, 512], F32R)
        wv = const.tile([P, 8, 256], F32R)
        bqk = const.tile([P, 4], F32)
        bv = const.tile([P, 256], F32)
        tri = const.tile([P, P], BF16)
        ebias = const.tile([P, 1], F32)
        qk = const.tile([P, 4, N], F32R)         # [p, {qq0,qq1,kk0,kk1}, n]
        vaug = const.tile([P, NJT, 4 * 65], BF16)
        wp = const.tile([P, 8, 1024], F32R)
        bp = const.tile([P, 1024], F32)
        ones_f32 = const.tile([1, 64], F32)
        ones_row = const.tile([1, 64], F32R)

        nc.sync.dma_start(bqk[:], d["b_qk"][:])
        nc.vector.memset(ebias[:], EBIAS)
        nc.vector.memset(vaug[:], 1.0)
        nc.vector.memset(ones_f32[:], 1.0)
        nc.vector.tensor_copy(ones_row[:], ones_f32[:])

        for _rep in range(repeats):
            # ---------------- QKV projection ----------------
            with tc.tile_pool(name="xp", bufs=1) as xp, \
                 tc.tile_pool(name="qkps", bufs=1, space="PSUM") as qkps, \
                 tc.tile_pool(name="vps", bufs=3, space="PSUM") as vps:
                xT = xp.tile([P, 8, N], F32R)
                for cc in range(8):
                    nc.sync.dma_start(xT[:, cc, :], d["xT"][cc])
                    nc.sync.dma_start(wqk[:, cc, :], d["w_qk"][cc])
                    nc.sync.dma_start(wv[:, cc, :], d["w_v"][cc])
                # deferred weight loads (needed later; keep xT DMAs first)
                nc.sync.dma_start(bv[:], d["b_v"][:])
                nc.sync.dma_start(tri[:], d["tri"][:])
                nc.sync.dma_start(wp[:], d["w_p"].rearrange("k p m -> p k m"))
                nc.sync.dma_start(bp[:], d["b_p"][:])

                # mb-outer / nb-inner keeps the stationary weight loaded
                # across 4 matmuls (one LDWEIGHTS per (mb, cc))
                for mb in range(4):
                    pss = [qkps.tile([P, 512], F32, tag=f"qk{nb}",
                                     name=f"qkps{nb}") for nb in range(NB)]
                    for cc in range(8):
                        for nb in range(NB):
                            nc.tensor.matmul(
                                pss[nb][:],
                                wqk[:, cc, P * mb:P * mb + P],
                                xT[:, cc, 512 * nb:512 * nb + 512],
                                start=(cc == 0), stop=(cc == 7),
                            )
                    for nb in range(NB):
                        nc.scalar.activation(
                            qk[:, mb, 512 * nb:512 * nb + 512], pss[nb][:],
                            AF.Identity, bias=bqk[:, mb:mb + 1], scale=1.0,
                        )
                for jt in range(NJT):
                    ps = vps.tile([P, 256], F32, tag="v")
                    for cc in range(8):
                        nc.tensor.matmul(
                            ps[:],
                            xT[:, cc, P * jt:P * jt + P],
                            wv[:, cc, :],
                            start=(cc == 0), stop=(cc == 7),
                        )
                    vview = vaug[:, jt, :].rearrange("p (h x) -> p h x", x=65)[:, :, 0:64]
                    nc.vector.tensor_add(
                        out=vview,
                        in0=ps[:].rearrange("p (h x) -> p h x", x=64),
                        in1=bv[:].rearrange("p (h x) -> p h x", x=64),
                    )

            # ---------------- attention + projection ----------------
            with tc.tile_pool(name="att", bufs=1) as att, \
                 tc.tile_pool(name="outp", bufs=1) as outp, \
                 tc.tile_pool(name="post", bufs=2) as post, \
                 tc.tile_pool(name="sps", bufs=3, space="PSUM") as sps, \
                 tc.tile_pool(name="avps", bufs=1, space="PSUM") as avps, \
                 tc.tile_pool(name="bcps", bufs=1, space="PSUM") as bcps, \
                 tc.tile_pool(name="pps", bufs=2, space="PSUM") as pps:
                for pair in range(2):
                    outTs = [outp.tile([64, N], F32, tag=f"outT{hp}", name=f"outT{hp}")
                             for hp in range(2)]
                    for m in range(NB):
                        njt = 4 * (m + 1)
                        expTs = [att.tile([P, NJT, 512], BF16, tag=f"expT{hp}",
                                          name=f"expT{hp}")
                                 for hp in range(2)]
                        for jt in range(njt):
                            pss = []
                            for hp in range(2):
                                lo = 64 * hp
                                ps_s = sps.tile([P, 512], F32, tag="sT")
                                nc.tensor.matmul(
                                    ps_s[:],
                                    qk[lo:lo + 64, 2 + pair, P * jt:P * jt + P],
                                    qk[lo:lo + 64, pair, 512 * m:512 * m + 512],
                                    start=True, stop=True,
                                )
                                pss.append(ps_s)
                            t = jt - 4 * m
                            for hp in range(2):
                                expT, ps_s = expTs[hp], pss[hp]
                                if t < 0:
                                    nc.scalar.activation(
                                        expT[:, jt, :], ps_s[:], AF.Exp,
                                        bias=ebias[:], scale=SCALE)
                                else:
                                    if t > 0:
                                        nc.vector.memset(expT[:, jt, 0:P * t], 0.0)
                                    nc.scalar.activation(
                                        expT[:, jt, P * t:512], ps_s[:, P * t:512],
                                        AF.Exp, bias=ebias[:], scale=SCALE)
                                    nc.vector.tensor_mul(
                                        out=expT[:, jt, P * t:P * t + P],
                                        in0=expT[:, jt, P * t:P * t + P], in1=tri[:])
                        for hp in range(2):
                            h = 2 * pair + hp
                            expT = expTs[hp]
                            ps_o = avps.tile([65, 512], F32, tag=f"av{hp}")
                            for jt in range(njt):
                                nc.tensor.matmul(
                                    ps_o[:],
                                    vaug[:, jt, 65 * h:65 * h + 65],
                                    expT[:, jt, :],
                                    start=(jt == 0), stop=(jt == njt - 1),
                                )
                            rec = post.tile([1, 512], F32R, tag="rec")
                            with nc.allow_low_precision(
                                    reason="f32r tag for fast broadcast matmul"):
                                nc.vector.reciprocal(rec[:], ps_o[64:65, :])
                            bcp = bcps.tile([64, 512], F32, tag="bc")
                            nc.tensor.matmul(bcp[:], ones_row[:], rec[:],
                                             start=True, stop=True)
                            bc = post.tile([64, 512], F32, tag="bc")
                            nc.vector.tensor_copy(bc[:], bcp[:])
                            nc.vector.tensor_mul(
                                out=outTs[hp][:, 512 * m:512 * m + 512],
                                in0=ps_o[0:64, :], in1=bc[:])
                    # staging + projection per head
                    for hp in range(2):
                        h = 2 * pair + hp
                        stage = post.tile([P, 8, P], F32R, tag="stage")
                        ov = outTs[hp][:].rearrange("p (q g) -> p g q", g=16)
                        for k in range(8):
                            nc.vector.tensor_copy(stage[0:64, k, :], ov[:, 2 * k, :])
                            nc.vector.tensor_copy(stage[64:P, k, :], ov[:, 2 * k + 1, :])
                        for mb2 in range(2):
                            psp = pps.tile([P, 512], F32, tag="proj")
                            for k in range(8):
                                nc.tensor.matmul(
                                    psp[:],
                                    stage[:, k, :],
                                    wp[:, k, 512 * mb2:512 * mb2 + 512],
                                    start=(k == 0), stop=(k == 7),
                                )
                            osb = post.tile([P, 512], F32, tag="osb")
                            nc.vector.tensor_add(
                                out=osb[:], in0=psp[:],
                                in1=bp[:, 512 * mb2:512 * mb2 + 512])
                            nc.sync.dma_start(
                                d["out"][P * h:P * h + P, 512 * mb2:512 * mb2 + 512],
                                osb[:])



def _fix_bir_for_walrus(bir: bytes) -> bytes:
    """Split multi-semaphore-wait instructions for walrus builds that
    support only one sync-wait command per instruction: extra waits are
    hoisted onto same-engine NoOps inserted immediately before.  ISA-class
    (custom Pool) instructions get ALL waits hoisted."""
    import json as _json

    d = _json.loads(bir)
    uid = [0]
    for fn in d["functions"]:
        for blk in fn["blocks"]:
            out = []
            for inst in blk["instructions"]:
                si = inst.get("sync_info")
                waits = (si or {}).get("on_wait") or []
                keep = 0 if "isa_opcode" in inst else 1
                if len(waits) > keep:
                    hoist, rest = waits[:len(waits) - keep], waits[len(waits) - keep:]
                    for w in hoist:
                        uid[0] += 1
                        out.append({
                            "name": f"I-wsplit-{uid[0]}",
                            "opcode": "NoOp",
                            "engine": inst["engine"],
                            "ins": [],
                            "outs": [],
                            "sync_info": {"on_wait": [w], "on_update": []},
                        })
                    si["on_wait"] = rest
                out.append(inst)
            blk["instructions"] = out
    return _json.dumps(d).encode()


_NC_CACHE = None


def build_bass(repeats: int = 1) -> bass.Bass:
    global _NC_CACHE
    if repeats == 1 and _NC_CACHE is not None:
        return _NC_CACHE
    nc = bass.Bass("TRN2", target_bir_lowering=False, debug=False,
                   enable_asserts=False, num_devices=8)
    d = {
        "xT": nc.dram_tensor("xT", [8, P, N], F32R, kind="ExternalInput").ap(),
        "w_qk": nc.dram_tensor("w_qk", [8, P, 512], F32R, kind="ExternalInput").ap(),
        "w_v": nc.dram_tensor("w_v", [8, P, 256], F32R, kind="ExternalInput").ap(),
        "b_qk": nc.dram_tensor("b_qk", [P, 4], F32, kind="ExternalInput").ap(),
        "b_v": nc.dram_tensor("b_v", [P, 256], F32, kind="ExternalInput").ap(),
        "w_p": nc.dram_tensor("w_p", [8, P, 1024], F32R, kind="ExternalInput").ap(),
        "b_p": nc.dram_tensor("b_p", [P, 1024], F32, kind="ExternalInput").ap(),
        "tri": nc.dram_tensor("tri", [P, P], BF16, kind="ExternalInput").ap(),
        "out": nc.dram_tensor("out", [512, 1024], F32, kind="ExternalOutput").ap(),
    }
    _emit(nc, d, repeats=repeats)
    _orig_to_json = nc.to_json_bytes
    nc.to_json_bytes = lambda: _fix_bir_for_walrus(_orig_to_json())
    if repeats == 1:
        _NC_CACHE = nc
    return nc


def _core_inputs(core: int, x, w_qkv, b_qkv, w_proj, b_proj) -> dict:
    import ml_dtypes

    b = core // 4
    h0 = 4 * (core % 4)
    xT = np.ascontiguousarray(x[b].T.reshape(8, P, N), np.float32)

    rows, brows = [], []
    for sec in (0, 1):                       # q section then k section
        for p in range(2):
            for e in range(2):
                h = h0 + 2 * p + e
                rows.append(w_qkv[sec * C + D * h: sec * C + D * h + D])
                brows.append(b_qkv[sec * C + D * h: sec * C + D * h + D])
    W_stack = np.concatenate(rows, 0)        # [512, 1024]
    w_qk = np.ascontiguousarray(W_stack.T.reshape(8, P, 512), np.float32)
    b_qk = np.ascontiguousarray(
        np.concatenate(brows, 0).reshape(4, P).T, np.float32)

    W_v4 = w_qkv[2 * C + D * h0: 2 * C + D * h0 + 256]
    w_v = np.ascontiguousarray(W_v4.T.reshape(8, P, 256), np.float32)
    b_v = np.ascontiguousarray(
        np.broadcast_to(b_qkv[2 * C + D * h0: 2 * C + D * h0 + 256], (P, 256)),
        np.float32)

    w_p = np.ascontiguousarray(w_proj.T.reshape(8, P, 1024), np.float32)
    b_p = np.ascontiguousarray(np.broadcast_to(b_proj, (P, 1024)), np.float32)
    tri = (np.arange(P)[None, :] >= np.arange(P)[:, None]).astype(ml_dtypes.bfloat16)
    return {"xT": xT, "w_qk": w_qk, "w_v": w_v, "b_qk": b_qk, "b_v": b_v,
            "w_p": w_p, "b_p": b_p, "tri": tri}


def _is_causal(mask: np.ndarray) -> bool:
    if mask.shape != (B, N, N):
        return False
    tril = np.tril(np.ones((N, N), bool))
    return bool(all(np.array_equal(mask[i], tril) for i in range(mask.shape[0])))


def _numpy_fallback(x, attention_mask, w_qkv, b_qkv, w_proj, b_proj):
    b, n, c = x.shape
    qkv = x @ w_qkv.T + b_qkv
    qkv = qkv.reshape(b, n, 3, H, D).transpose(2, 0, 3, 1, 4)
    q, k, v = qkv[0], qkv[1], qkv[2]
    dots = np.einsum("bhid,bhjd->bhij", q, k) * SCALE
    mask_value = -np.finfo(dots.dtype).max
    dots = np.where(attention_mask[:, None, :, :], dots, mask_value)
    dots = dots - dots.max(axis=-1, keepdims=True)
    e = np.exp(dots)
    attn = e / e.sum(axis=-1, keepdims=True)
    out = np.einsum("bhij,bhjd->bhid", attn, v)
    out = out.reshape(b, n, c)
    return (out @ w_proj.T + b_proj).astype(np.float32)


def kernel(**inputs) -> np.ndarray:
    x = np.asarray(inputs["x"], np.float32)
    mask = np.asarray(inputs["attention_mask"])
    w_qkv = np.asarray(inputs["w_qkv"], np.float32)
    b_qkv = np.asarray(inputs["b_qkv"], np.float32)
    w_proj = np.asarray(inputs["w_proj"], np.float32)
    b_proj = np.asarray(inputs["b_proj"], np.float32)

    if not _is_causal(mask):
        return _numpy_fallback(x, mask, w_qkv, b_qkv, w_proj, b_proj)

    nc = build_bass()
    in_maps = [_core_inputs(c, x, w_qkv, b_qkv, w_proj, b_proj)
               for c in range(8)]
    res = run_bass_kernel_spmd(nc, in_maps, core_ids=list(range(8)))
    out = np.empty((B, N, C), np.float32)
    for c in range(8):
        b = c // 4
        h0 = 4 * (c % 4)
        out[b, P * h0:P * h0 + 512, :] = res.results[c]["out"]
    return out



# revision 12
# speedup vs baseline: 1.4676x; 1.4676x over previous
"""Trainium2 Bass kernel for nn_MaskedAttention (B=2, N=2048, C=1024, H=16).

Sharding: batch x head-group over 8 cores (core c -> batch c//4, heads
4*(c%4)..4*(c%4)+3).  The reference's "faithful" head-scrambled reshape
means each head's output occupies a contiguous 128-row block of the
pre-projection matrix, so the output projection is row-parallel across
heads and needs no cross-core reduction.

Pipeline highlights (vs the straightforward version):
  - all matmul operands bf16 (halves input DMA, enables fast weight load);
    fp32 only in PSUM accumulators and the softmax denominator path.
  - k-bias dropped (softmax-invariant: it shifts each score row by a
    per-row constant); v-bias folded into an effective projection bias on
    the host (bp_eff = b_proj + w_proj @ tile(bv_h)); only q keeps its bias.
  - scores computed transposed sT[j,i] per 128x512 tile for both heads of
    a pair at once (row groups 0/64 -> concurrent on HW); exp reads the
    two heads' tiles as one [128,1024] PSUM-spanning activation.
  - AV uses the augmented-[V|1] stationary trick: row 64 of the PSUM
    output is the softmax denominator for free.
  - reciprocal on DVE, partition-broadcast on GPSIMD (Pool), diagonal-tile
    memsets on Pool; normalization multiply writes directly in the
    head-scrambled projection staging layout (no separate copies).
  - QKV / V / projection chains are interleaved into the attention stream
    as PE "filler" work so the PE never idles while ACT grinds exp.
"""

import numpy as np

import concourse.bass as bass
import concourse.mybir as mybir
from concourse import tile
from concourse.bass_utils import run_bass_kernel_spmd

B, N, C, H = 2, 2048, 1024, 16
D = C // H                 # 64
SCALE = D ** -0.5
EBIAS = -20.0
P = 128
NB = N // 512              # 4 n blocks
NJT = N // P               # 16 j tiles
F32 = mybir.dt.float32
BF16 = mybir.dt.bfloat16
AF = mybir.ActivationFunctionType


def _emit(nc: bass.Bass, d: dict, repeats: int = 1):
    from contextlib import ExitStack

    with tile.TileContext(nc) as tc, ExitStack() as ctx:
        const = ctx.enter_context(tc.tile_pool(name="const", bufs=1))
        wqk = const.tile([P, 8, 512], BF16)
        wv = const.tile([P, 8, 256], BF16)
        bq = const.tile([P, 2], F32)
        tri2 = const.tile([P, 2, P], BF16)
        ebias = const.tile([P, 1], F32)
        qk = const.tile([P, 4, N], BF16)         # [p, {qq0,qq1,kk0,kk1}, n]
        xT = const.tile([P, 8, N], BF16)
        vaug = const.tile([P, NJT, 4 * 65], BF16)
        wp = const.tile([P, 8, 1024], BF16)
        bp = const.tile([P, 4, 1024], BF16)      # per-head effective proj bias

        nc.vector.memset(ebias[:], EBIAS)
        nc.gpsimd.memset(vaug[:], 1.0)

        for _rep in range(repeats):
            with tc.tile_pool(name="sps", bufs=2, space="PSUM") as sps, \
                 tc.tile_pool(name="avps", bufs=1, space="PSUM") as avps, \
                 tc.tile_pool(name="fps", bufs=2, space="PSUM") as fps, \
                 tc.tile_pool(name="att", bufs=2) as att, \
                 tc.tile_pool(name="stg", bufs=2) as stg, \
                 tc.tile_pool(name="post", bufs=3) as post:
                # ---------- input DMAs ----------
                nc.sync.dma_start(
                    wqk[:, 0:2, :], d["w_qk"][0:2].rearrange("c p w -> p c w"))
                nc.sync.dma_start(
                    xT[:, 0:2, 0:512],
                    d["xT"][0, 0:2].rearrange("c p w -> p c w"))
                nc.sync.dma_start(
                    wqk[:, 2:8, :], d["w_qk"][2:8].rearrange("c p w -> p c w"))
                nc.sync.dma_start(
                    xT[:, 2:8, 0:512],
                    d["xT"][0, 2:8].rearrange("c p w -> p c w"))
                nc.sync.dma_start(bq[:], d["b_q"][:])
                nc.sync.dma_start(wv[:], d["w_v"].rearrange("c p w -> p c w"))
                for nb in range(1, NB):
                    nc.sync.dma_start(
                        xT[:, :, 512 * nb:512 * nb + 512],
                        d["xT"][nb].rearrange("c p w -> p c w"))
                nc.sync.dma_start(tri2[:], d["tri2"][:])
                nc.sync.dma_start(wp[:], d["w_p"].rearrange("c p w -> p c w"))
                nc.sync.dma_start(bp[:], d["b_p"].rearrange("h p w -> p h w"))

                # ---------- PE work-chunk emitters ----------
                def emit_qk_chain(mb, nb):
                    # q rows (mb 0/1) carry the bias; k rows (mb 2/3) don't
                    ps = fps.tile([P, 512], F32, tag="f")
                    for cc in range(8):
                        nc.tensor.matmul(
                            ps[:],
                            wqk[:, cc, P * mb:P * mb + P],
                            xT[:, cc, 512 * nb:512 * nb + 512],
                            start=(cc == 0), stop=(cc == 7),
                        )
                    dst = qk[:, mb, 512 * nb:512 * nb + 512]
                    if mb < 2:
                        nc.vector.tensor_scalar_add(dst, ps[:], bq[:, mb:mb + 1])
                    else:
                        nc.vector.tensor_copy(dst, ps[:])

                def emit_v_chain(jt):
                    ps = fps.tile([P, 512], F32, tag="f")
                    for cc in range(8):
                        nc.tensor.matmul(
                            ps[:, 0:256],
                            xT[:, cc, P * jt:P * jt + P],
                            wv[:, cc, :],
                            start=(cc == 0), stop=(cc == 7),
                        )
                    nc.vector.tensor_copy(
                        out=vaug[:, jt, :].rearrange(
                            "p (h x) -> p h x", x=65)[:, :, 0:64],
                        in_=ps[:, 0:256].rearrange("p (h x) -> p h x", x=64),
                    )

                def emit_proj_chunk(pair, hp, mb2, stage, osb):
                    h = 2 * pair + hp
                    ps = fps.tile([P, 512], F32, tag="f")
                    for k in range(8):
                        nc.tensor.matmul(
                            ps[:],
                            stage[:, k, :],
                            wp[:, k, 512 * mb2:512 * mb2 + 512],
                            start=(k == 0), stop=(k == 7),
                        )
                    nc.vector.tensor_add(
                        out=osb[:, 512 * mb2:512 * mb2 + 512], in0=ps[:],
                        in1=bp[:, h, 512 * mb2:512 * mb2 + 512])
                    if mb2 == 1:
                        nc.sync.dma_start(d["out"][P * h:P * h + P, :], osb[:])

                # ---------- prologue: q/k for pair 0, v for jt 0-3 ----------
                for nb in range(NB):
                    emit_qk_chain(0, nb)
                    emit_qk_chain(2, nb)
                for jt in range(4):
                    emit_v_chain(jt)

                # ---------- attention with woven PE fillers ----------
                def make_fillers(pair):
                    fills = []
                    if pair == 0:
                        for jt in range(4, NJT):
                            fills.append(("v", jt))
                        for nb in range(NB):
                            fills.append(("qk", 1, nb))
                            fills.append(("qk", 3, nb))
                    return fills

                proj_state = {}

                def run_filler(f):
                    if f[0] == "v":
                        emit_v_chain(f[1])
                    elif f[0] == "qk":
                        emit_qk_chain(f[1], f[2])
                    else:
                        _, pair, hp, mb2 = f
                        key = (pair, hp)
                        if key not in proj_state:
                            proj_state[key] = post.tile([P, 1024], F32, tag="osb",
                                                        name=f"osb{pair}{hp}")
                        emit_proj_chunk(pair, hp, mb2, stages[key],
                                        proj_state[key])

                stages = {}
                for pair in range(2):
                    fillers = make_fillers(pair)
                    if pair == 1:
                        for hp in range(2):
                            for mb2 in range(2):
                                fillers.append(("proj", 0, hp, mb2))
                    fi = 0
                    nbatch = sum(4 * (m + 1) for m in range(NB))
                    for hp in range(2):
                        stages[(pair, hp)] = stg.tile([P, 8, P], BF16, tag=f"stage{hp}", name=f"stage{pair}{hp}")
                    bi = 0
                    for m in range(NB):
                        njt = 4 * (m + 1)
                        expT = att.tile([P, NJT, 2, 512], BF16, tag="expT")
                        pss = [avps.tile([65, 512], F32, tag=f"av{hp}",
                                        name=f"av{hp}")
                               for hp in range(2)]

                        def emit_scores(jt):
                            s = sps.tile([P, 1024], F32, tag="s")
                            for hp in range(2):
                                lo = 64 * hp
                                nc.tensor.matmul(
                                    s[:, 512 * hp:512 * hp + 512],
                                    qk[lo:lo + 64, 2 + pair,
                                       P * jt:P * jt + P],
                                    qk[lo:lo + 64, pair,
                                       512 * m:512 * m + 512],
                                    start=True, stop=True,
                                )
                            t = jt - 4 * m
                            ex = expT[:, jt, :, :]
                            if t < 0:
                                nc.scalar.activation(
                                    ex.rearrange("p h w -> p (h w)"),
                                    s[:], AF.Exp, bias=ebias[:], scale=SCALE)
                            else:
                                if t > 0:
                                    nc.gpsimd.memset(ex[:, :, 0:P * t], 0.0)
                                nc.scalar.activation(
                                    ex[:, :, P * t:512],
                                    s[:].rearrange(
                                        "p (h w) -> p h w", h=2)[:, :, P * t:512],
                                    AF.Exp, bias=ebias[:], scale=SCALE)
                                nc.vector.tensor_mul(
                                    out=ex[:, :, P * t:P * t + P],
                                    in0=ex[:, :, P * t:P * t + P], in1=tri2[:])

                        def emit_av(jt):
                            for hp in range(2):
                                h = 2 * pair + hp
                                nc.tensor.matmul(
                                    pss[hp][:],
                                    vaug[:, jt, 65 * h:65 * h + 65],
                                    expT[:, jt, hp, :],
                                    start=(jt == 0), stop=(jt == njt - 1),
                                )

                        # software-pipelined: scores run two tiles ahead of AV
                        emit_scores(0)
                        if njt > 1:
                            emit_scores(1)
                        for jt in range(njt):
                            if jt + 2 < njt:
                                emit_scores(jt + 2)
                            emit_av(jt)
                            bi += 1
                            # weave PE filler chunks evenly across batches
                            if fi < len(fillers) and bi * len(fillers) >= \
                                    nbatch * (fi + 1):
                                run_filler(fillers[fi])
                                fi += 1
                        # normalization directly into the staged layout
                        for hp in range(2):
                            ps_o = pss[hp]
                            rec = post.tile([1, 512], F32, tag="rec")
                            nc.vector.reciprocal(rec[:], ps_o[64:65, :])
                            bc = post.tile([64, 512], F32, tag="bc")
                            nc.gpsimd.dma_start(
                                bc[:],
                                rec[0:1, None, :].broadcast_to((1, 64, 512)))
                            stage = stages[(pair, hp)]
                            for e in range(2):
                                nc.vector.tensor_mul(
                                    out=stage[64 * e:64 * e + 64, :,
                                              32 * m:32 * m + 32],
                                    in0=ps_o[0:64, :].rearrange(
                                        "p (q k e) -> p k q e",
                                        k=8, e=2)[:, :, :, e],
                                    in1=bc[:, :].rearrange(
                                        "p (q k e) -> p k q e",
                                        k=8, e=2)[:, :, :, e],
                                )
                            if pair == 1 and m == NB - 1:
                                # tail: project this head as soon as staged
                                run_filler(("proj", 1, hp, 0))
                                run_filler(("proj", 1, hp, 1))
                    while fi < len(fillers):
                        run_filler(fillers[fi])
                        fi += 1


def _fix_bir_for_walrus(bir: bytes) -> bytes:
    """Split multi-semaphore-wait instructions for walrus builds that
    support only one sync-wait command per instruction: extra waits are
    hoisted onto same-engine NoOps inserted immediately before.  ISA-class
    (custom Pool) instructions get ALL waits hoisted."""
    import json as _json

    d = _json.loads(bir)
    uid = [0]
    for fn in d["functions"]:
        for blk in fn["blocks"]:
            out = []
            for inst in blk["instructions"]:
                si = inst.get("sync_info")
                waits = (si or {}).get("on_wait") or []
                keep = 0 if "isa_opcode" in inst else 1
                if len(waits) > keep:
                    hoist, rest = waits[:len(waits) - keep], waits[len(waits) - keep:]
                    for w in hoist:
                        uid[0] += 1
                        out.append({
                            "name": f"I-wsplit-{uid[0]}",
                            "opcode": "NoOp",
                            "engine": inst["engine"],
                            "ins": [],
                            "outs": [],
                            "sync_info": {"on_wait": [w], "on_update": []},
                        })
                    si["on_wait"] = rest
                out.append(inst)
            blk["instructions"] = out
    return _json.dumps(d).encode()


_NC_CACHE = None


def build_bass(repeats: int = 1) -> bass.Bass:
    global _NC_CACHE
    if repeats == 1 and _NC_CACHE is not None:
        return _NC_CACHE
    nc = bass.Bass("TRN2", target_bir_lowering=False, debug=False,
                   enable_asserts=False, num_devices=8)
    d = {
        "xT": nc.dram_tensor("xT", [NB, 8, P, 512], BF16, kind="ExternalInput").ap(),
        "w_qk": nc.dram_tensor("w_qk", [8, P, 512], BF16, kind="ExternalInput").ap(),
        "w_v": nc.dram_tensor("w_v", [8, P, 256], BF16, kind="ExternalInput").ap(),
        "b_q": nc.dram_tensor("b_q", [P, 2], F32, kind="ExternalInput").ap(),
        "w_p": nc.dram_tensor("w_p", [8, P, 1024], BF16, kind="ExternalInput").ap(),
        "b_p": nc.dram_tensor("b_p", [4, P, 1024], BF16, kind="ExternalInput").ap(),
        "tri2": nc.dram_tensor("tri2", [P, 2, P], BF16, kind="ExternalInput").ap(),
        "out": nc.dram_tensor("out", [512, 1024], F32, kind="ExternalOutput").ap(),
    }
    _emit(nc, d, repeats=repeats)
    _orig_to_json = nc.to_json_bytes
    nc.to_json_bytes = lambda: _fix_bir_for_walrus(_orig_to_json())
    if repeats == 1:
        _NC_CACHE = nc
    return nc


def _core_inputs(core: int, x, w_qkv, b_qkv, w_proj, b_proj) -> dict:
    import ml_dtypes

    BF = ml_dtypes.bfloat16
    b = core // 4
    h0 = 4 * (core % 4)
    xT = np.ascontiguousarray(
        x[b].T.reshape(8, P, NB, 512).transpose(2, 0, 1, 3), BF)

    rows, brows = [], []
    for sec in (0, 1):                       # q section then k section
        for p in range(2):
            for e in range(2):
                h = h0 + 2 * p + e
                rows.append(w_qkv[sec * C + D * h: sec * C + D * h + D])
                brows.append(b_qkv[sec * C + D * h: sec * C + D * h + D])
    W_stack = np.concatenate(rows, 0)        # [512, 1024]
    w_qk = np.ascontiguousarray(W_stack.T.reshape(8, P, 512), BF)
    # only the q biases (first two 128-row groups); k bias is dropped
    # (it shifts every score in a row by the same amount -> softmax-inv).
    b_q = np.ascontiguousarray(
        np.concatenate(brows[:4], 0).reshape(2, P).T, np.float32)

    W_v4 = w_qkv[2 * C + D * h0: 2 * C + D * h0 + 256]
    w_v = np.ascontiguousarray(W_v4.T.reshape(8, P, 256), BF)

    w_p = np.ascontiguousarray(w_proj.T.reshape(8, P, 1024), BF)
    # effective proj bias per head: b_proj + w_proj @ tile(bv_h, 16)
    b_p = np.empty((4, P, 1024), np.float32)
    for hh in range(4):
        bv = b_qkv[2 * C + D * (h0 + hh): 2 * C + D * (h0 + hh) + D]
        eff = b_proj + w_proj @ np.tile(bv, 16)
        b_p[hh] = np.broadcast_to(eff, (P, 1024))
    b_p = np.ascontiguousarray(b_p, BF)
    tri = (np.arange(P)[None, :] >= np.arange(P)[:, None]).astype(BF)
    tri2 = np.ascontiguousarray(
        np.broadcast_to(tri[:, None, :], (P, 2, P)), BF)
    return {"xT": xT, "w_qk": w_qk, "w_v": w_v, "b_q": b_q,
            "w_p": w_p, "b_p": b_p, "tri2": tri2}


def _is_causal(mask: np.ndarray) -> bool:
    if mask.shape != (B, N, N):
        return False
    tril = np.tril(np.ones((N, N), bool))
    return bool(all(np.array_equal(mask[i], tril) for i in range(mask.shape[0])))


def _numpy_fallback(x, attention_mask, w_qkv, b_qkv, w_proj, b_proj):
    b, n, c = x.shape
    qkv = x @ w_qkv.T + b_qkv
    qkv = qkv.reshape(b, n, 3, H, D).transpose(2, 0, 3, 1, 4)
    q, k, v = qkv[0], qkv[1], qkv[2]
    dots = np.einsum("bhid,bhjd->bhij", q, k) * SCALE
    mask_value = -np.finfo(dots.dtype).max
    dots = np.where(attention_mask[:, None, :, :], dots, mask_value)
    dots = dots - dots.max(axis=-1, keepdims=True)
    e = np.exp(dots)
    attn = e / e.sum(axis=-1, keepdims=True)
    out = np.einsum("bhij,bhjd->bhid", attn, v)
    out = out.reshape(b, n, c)
    return (out @ w_proj.T + b_proj).astype(np.float32)


def kernel(**inputs) -> np.ndarray:
    x = np.asarray(inputs["x"], np.float32)
    mask = np.asarray(inputs["attention_mask"])
    w_qkv = np.asarray(inputs["w_qkv"], np.float32)
    b_qkv = np.asarray(inputs["b_qkv"], np.float32)
    w_proj = np.asarray(inputs["w_proj"], np.float32)
    b_proj = np.asarray(inputs["b_proj"], np.float32)

    if not _is_causal(mask):
        return _numpy_fallback(x, mask, w_qkv, b_qkv, w_proj, b_proj)

    nc = build_bass()
    in_maps = [_core_inputs(c, x, w_qkv, b_qkv, w_proj, b_proj)
               for c in range(8)]
    res = run_bass_kernel_spmd(nc, in_maps, core_ids=list(range(8)))
    out = np.empty((B, N, C), np.float32)
    for c in range(8):
        b = c // 4
        h0 = 4 * (c % 4)
        out[b, P * h0:P * h0 + 512, :] = res.results[c]["out"]
    return out


# revision 14
# speedup vs baseline: 1.6307x; 1.1111x over previous
"""Trainium2 Bass kernel for nn_MaskedAttention (B=2, N=2048, C=1024, H=16).

Sharding: batch x head-group over 8 cores (core c -> batch c//4, heads
4*(c%4)..4*(c%4)+3).  The reference's "faithful" head-scrambled reshape
means each head's output occupies a contiguous 128-row block of the
pre-projection matrix, so the output projection is row-parallel across
heads and needs no cross-core reduction.

Pipeline highlights (vs the straightforward version):
  - all matmul operands bf16 (halves input DMA, enables fast weight load);
    fp32 only in PSUM accumulators and the softmax denominator path.
  - k-bias dropped (softmax-invariant: it shifts each score row by a
    per-row constant); v-bias folded into an effective projection bias on
    the host (bp_eff = b_proj + w_proj @ tile(bv_h)); only q keeps its bias.
  - scores computed transposed sT[j,i] per 128x512 tile for both heads of
    a pair at once (row groups 0/64 -> concurrent on HW); exp reads the
    two heads' tiles as one [128,1024] PSUM-spanning activation.
  - AV uses the augmented-[V|1] stationary trick: row 64 of the PSUM
    output is the softmax denominator for free.
  - reciprocal on DVE, partition-broadcast on GPSIMD (Pool), diagonal-tile
    memsets on Pool; normalization multiply writes directly in the
    head-scrambled projection staging layout (no separate copies).
  - QKV / V / projection chains are interleaved into the attention stream
    as PE "filler" work so the PE never idles while ACT grinds exp.
"""

import numpy as np

import concourse.bass as bass
import concourse.mybir as mybir
from concourse import tile
from concourse.bass_utils import run_bass_kernel_spmd

B, N, C, H = 2, 2048, 1024, 16
D = C // H                 # 64
SCALE = D ** -0.5
EBIAS = -20.0
P = 128
NB = N // 512              # 4 n blocks
NJT = N // P               # 16 j tiles
F32 = mybir.dt.float32
BF16 = mybir.dt.bfloat16
AF = mybir.ActivationFunctionType


def _emit(nc: bass.Bass, d: dict, repeats: int = 1):
    from contextlib import ExitStack

    with tile.TileContext(nc) as tc, ExitStack() as ctx:
        const = ctx.enter_context(tc.tile_pool(name="const", bufs=1))
        wqk = const.tile([P, 8, 512], BF16)
        wv = const.tile([P, 8, 256], BF16)
        bq = const.tile([P, 2], F32)
        tri2 = const.tile([P, 2, P], BF16)
        ebias = const.tile([P, 1], F32)
        qk = const.tile([P, 4, N], BF16)         # [p, {qq0,qq1,kk0,kk1}, n]
        xT = const.tile([P, 8, N], BF16)
        vaug = const.tile([P, NJT, 4 * 65], BF16)
        wp = const.tile([P, 8, 1024], BF16)
        bp = const.tile([P, 4, 1024], BF16)      # per-head effective proj bias

        nc.vector.memset(ebias[:], EBIAS)
        nc.gpsimd.memset(vaug[:], 1.0)

        for _rep in range(repeats):
            with tc.tile_pool(name="sps", bufs=2, space="PSUM") as sps, \
                 tc.tile_pool(name="avps", bufs=1, space="PSUM") as avps, \
                 tc.tile_pool(name="fps", bufs=2, space="PSUM") as fps, \
                 tc.tile_pool(name="att", bufs=2) as att, \
                 tc.tile_pool(name="stg", bufs=2) as stg, \
                 tc.tile_pool(name="post", bufs=3) as post:
                # ---------- input DMAs ----------
                nc.sync.dma_start(
                    wqk[:, 0:2, :], d["w_qk"][0:2].rearrange("c p w -> p c w"))
                nc.sync.dma_start(
                    xT[:, 0:2, 0:512],
                    d["xT"][0, 0:2].rearrange("c p w -> p c w"))
                nc.sync.dma_start(
                    wqk[:, 2:8, :], d["w_qk"][2:8].rearrange("c p w -> p c w"))
                nc.sync.dma_start(
                    xT[:, 2:8, 0:512],
                    d["xT"][0, 2:8].rearrange("c p w -> p c w"))
                nc.sync.dma_start(bq[:], d["b_q"][:])
                nc.sync.dma_start(wv[:], d["w_v"].rearrange("c p w -> p c w"))
                for nb in range(1, NB):
                    nc.sync.dma_start(
                        xT[:, :, 512 * nb:512 * nb + 512],
                        d["xT"][nb].rearrange("c p w -> p c w"))
                nc.sync.dma_start(tri2[:], d["tri2"][:])
                nc.sync.dma_start(wp[:], d["w_p"].rearrange("c p w -> p c w"))
                nc.sync.dma_start(bp[:], d["b_p"].rearrange("h p w -> p h w"))

                # ---------- PE work-chunk emitters ----------
                def emit_qk_chain(mb, nb):
                    # q rows (mb 0/1) carry the bias; k rows (mb 2/3) don't
                    ps = fps.tile([P, 512], F32, tag="f")
                    for cc in range(8):
                        nc.tensor.matmul(
                            ps[:],
                            wqk[:, cc, P * mb:P * mb + P],
                            xT[:, cc, 512 * nb:512 * nb + 512],
                            start=(cc == 0), stop=(cc == 7),
                        )
                    dst = qk[:, mb, 512 * nb:512 * nb + 512]
                    if mb < 2:
                        nc.vector.tensor_scalar_add(dst, ps[:], bq[:, mb:mb + 1])
                    else:
                        nc.vector.tensor_copy(dst, ps[:])

                def emit_v_chain(jt):
                    ps = fps.tile([P, 512], F32, tag="f")
                    for cc in range(8):
                        nc.tensor.matmul(
                            ps[:, 0:256],
                            xT[:, cc, P * jt:P * jt + P],
                            wv[:, cc, :],
                            start=(cc == 0), stop=(cc == 7),
                        )
                    nc.vector.tensor_copy(
                        out=vaug[:, jt, :].rearrange(
                            "p (h x) -> p h x", x=65)[:, :, 0:64],
                        in_=ps[:, 0:256].rearrange("p (h x) -> p h x", x=64),
                    )

                def emit_proj_chunk(pair, hp, mb2, stage, osb):
                    h = 2 * pair + hp
                    ps = fps.tile([P, 512], F32, tag="f")
                    for k in range(8):
                        nc.tensor.matmul(
                            ps[:],
                            stage[:, k, :],
                            wp[:, k, 512 * mb2:512 * mb2 + 512],
                            start=(k == 0), stop=(k == 7),
                        )
                    nc.vector.tensor_add(
                        out=osb[:, 512 * mb2:512 * mb2 + 512], in0=ps[:],
                        in1=bp[:, h, 512 * mb2:512 * mb2 + 512])
                    nc.scalar.dma_start(
                        d["out"][P * h:P * h + P, 512 * mb2:512 * mb2 + 512],
                        osb[:, 512 * mb2:512 * mb2 + 512])

                # ---------- prologue: just enough q/k for (pair0, m=0) ----
                emit_qk_chain(0, 0)
                emit_qk_chain(2, 0)

                # ---------- attention with deadline-woven PE fillers ------
                proj_state = {}

                def run_filler(f):
                    if f[0] == "v":
                        emit_v_chain(f[1])
                    elif f[0] == "qk":
                        emit_qk_chain(f[1], f[2])
                    else:
                        _, pair, hp, mb2 = f
                        key = (pair, hp)
                        if key not in proj_state:
                            proj_state[key] = post.tile([P, 1024], F32, tag="osb",
                                                        name=f"osb{pair}{hp}")
                        emit_proj_chunk(pair, hp, mb2, stages[key],
                                        proj_state[key])

                def make_fillers(pair):
                    # ordered so force-draining to a deadline key is safe
                    fills = []
                    if pair == 0:
                        for jt in range(0, 4):
                            fills.append(("v", jt))
                        for nb in range(1, NB):
                            fills.append(("qk", 0, nb))
                            fills.append(("qk", 2, nb))
                            for jt in range(4 * nb, 4 * nb + 4):
                                fills.append(("v", jt))
                        for nb in range(NB):
                            fills.append(("qk", 1, nb))
                            fills.append(("qk", 3, nb))
                    else:
                        for hp in range(2):
                            for mb2 in range(2):
                                fills.append(("proj", 0, hp, mb2))
                    return fills

                stages = {}
                for pair in range(2):
                    fillers = make_fillers(pair)
                    fi = 0
                    nbatch = sum(4 * (m + 1) for m in range(NB))

                    def drain_until(fkey):
                        # emit fillers up to and including fkey (if pending)
                        nonlocal fi
                        if fkey not in fillers[fi:]:
                            return
                        stop = fillers.index(fkey, fi)
                        while fi <= stop:
                            run_filler(fillers[fi])
                            fi += 1
                    for hp in range(2):
                        stages[(pair, hp)] = stg.tile([P, 8, P], BF16, tag=f"stage{hp}", name=f"stage{pair}{hp}")
                    bi = 0
                    for m in range(NB):
                        njt = 4 * (m + 1)
                        expT = att.tile([P, NJT, 2, 512], BF16, tag="expT")
                        pss = [avps.tile([65, 512], F32, tag=f"av{hp}",
                                        name=f"av{hp}")
                               for hp in range(2)]

                        def emit_scores(jt):
                            s = sps.tile([P, 1024], F32, tag="s")
                            for hp in range(2):
                                lo = 64 * hp
                                nc.tensor.matmul(
                                    s[:, 512 * hp:512 * hp + 512],
                                    qk[lo:lo + 64, 2 + pair,
                                       P * jt:P * jt + P],
                                    qk[lo:lo + 64, pair,
                                       512 * m:512 * m + 512],
                                    start=True, stop=True,
                                )
                            t = jt - 4 * m
                            ex = expT[:, jt, :, :]
                            if t < 0:
                                nc.scalar.activation(
                                    ex.rearrange("p h w -> p (h w)"),
                                    s[:], AF.Exp, bias=ebias[:], scale=SCALE)
                            else:
                                if t > 0:
                                    nc.gpsimd.memset(ex[:, :, 0:P * t], 0.0)
                                nc.scalar.activation(
                                    ex[:, :, P * t:512],
                                    s[:].rearrange(
                                        "p (h w) -> p h w", h=2)[:, :, P * t:512],
                                    AF.Exp, bias=ebias[:], scale=SCALE)
                                nc.vector.tensor_mul(
                                    out=ex[:, :, P * t:P * t + P],
                                    in0=ex[:, :, P * t:P * t + P], in1=tri2[:])

                        def emit_av(jt):
                            for hp in range(2):
                                h = 2 * pair + hp
                                nc.tensor.matmul(
                                    pss[hp][:],
                                    vaug[:, jt, 65 * h:65 * h + 65],
                                    expT[:, jt, hp, :],
                                    start=(jt == 0), stop=(jt == njt - 1),
                                )

                        # deadlines: q/k chains this m-block's scores need
                        if pair == 0 and m > 0:
                            drain_until(("qk", 2, m))
                        # software-pipelined: scores run two tiles ahead of AV
                        emit_scores(0)
                        if njt > 1:
                            emit_scores(1)
                        for jt in range(njt):
                            if jt + 2 < njt:
                                emit_scores(jt + 2)
                            if pair == 0:
                                drain_until(("v", jt))
                            emit_av(jt)
                            bi += 1
                            # weave PE filler chunks evenly across batches
                            if fi < len(fillers) and bi * len(fillers) >= \
                                    nbatch * (fi + 1):
                                run_filler(fillers[fi])
                                fi += 1
                        # normalization directly into the staged layout
                        for hp in range(2):
                            ps_o = pss[hp]
                            rec = post.tile([1, 512], F32, tag="rec")
                            nc.vector.reciprocal(rec[:], ps_o[64:65, :])
                            bc = post.tile([64, 512], F32, tag="bc")
                            nc.gpsimd.dma_start(
                                bc[:],
                                rec[0:1, None, :].broadcast_to((1, 64, 512)))
                            stage = stages[(pair, hp)]
                            for e in range(2):
                                nc.vector.tensor_mul(
                                    out=stage[64 * e:64 * e + 64, :,
                                              32 * m:32 * m + 32],
                                    in0=ps_o[0:64, :].rearrange(
                                        "p (q k e) -> p k q e",
                                        k=8, e=2)[:, :, :, e],
                                    in1=bc[:, :].rearrange(
                                        "p (q k e) -> p k q e",
                                        k=8, e=2)[:, :, :, e],
                                )
                            if pair == 1 and m == NB - 1:
                                # tail: project this head as soon as staged
                                run_filler(("proj", 1, hp, 0))
                                run_filler(("proj", 1, hp, 1))
                    while fi < len(fillers):
                        run_filler(fillers[fi])
                        fi += 1


def _fix_bir_for_walrus(bir: bytes) -> bytes:
    """Split multi-semaphore-wait instructions for walrus builds that
    support only one sync-wait command per instruction: extra waits are
    hoisted onto same-engine NoOps inserted immediately before.  ISA-class
    (custom Pool) instructions get ALL waits hoisted."""
    import json as _json

    d = _json.loads(bir)
    uid = [0]
    for fn in d["functions"]:
        for blk in fn["blocks"]:
            out = []
            for inst in blk["instructions"]:
                si = inst.get("sync_info")
                waits = (si or {}).get("on_wait") or []
                keep = 0 if "isa_opcode" in inst else 1
                if len(waits) > keep:
                    hoist, rest = waits[:len(waits) - keep], waits[len(waits) - keep:]
                    for w in hoist:
                        uid[0] += 1
                        out.append({
                            "name": f"I-wsplit-{uid[0]}",
                            "opcode": "NoOp",
                            "engine": inst["engine"],
                            "ins": [],
                            "outs": [],
                            "sync_info": {"on_wait": [w], "on_update": []},
                        })
                    si["on_wait"] = rest
                out.append(inst)
            blk["instructions"] = out
    return _json.dumps(d).encode()


_NC_CACHE = None


def build_bass(repeats: int = 1) -> bass.Bass:
    global _NC_CACHE
    if repeats == 1 and _NC_CACHE is not None:
        return _NC_CACHE
    nc = bass.Bass("TRN2", target_bir_lowering=False, debug=False,
                   enable_asserts=False, num_devices=8)
    d = {
        "xT": nc.dram_tensor("xT", [NB, 8, P, 512], BF16, kind="ExternalInput").ap(),
        "w_qk": nc.dram_tensor("w_qk", [8, P, 512], BF16, kind="ExternalInput").ap(),
        "w_v": nc.dram_tensor("w_v", [8, P, 256], BF16, kind="ExternalInput").ap(),
        "b_q": nc.dram_tensor("b_q", [P, 2], F32, kind="ExternalInput").ap(),
        "w_p": nc.dram_tensor("w_p", [8, P, 1024], BF16, kind="ExternalInput").ap(),
        "b_p": nc.dram_tensor("b_p", [4, P, 1024], BF16, kind="ExternalInput").ap(),
        "tri2": nc.dram_tensor("tri2", [P, 2, P], BF16, kind="ExternalInput").ap(),
        "out": nc.dram_tensor("out", [512, 1024], F32, kind="ExternalOutput").ap(),
    }
    _emit(nc, d, repeats=repeats)
    _orig_to_json = nc.to_json_bytes
    nc.to_json_bytes = lambda: _fix_bir_for_walrus(_orig_to_json())
    if repeats == 1:
        _NC_CACHE = nc
    return nc


def _core_inputs(core: int, x, w_qkv, b_qkv, w_proj, b_proj) -> dict:
    import ml_dtypes

    BF = ml_dtypes.bfloat16
    b = core // 4
    h0 = 4 * (core % 4)
    xT = np.ascontiguousarray(
        x[b].T.reshape(8, P, NB, 512).transpose(2, 0, 1, 3), BF)

    rows, brows = [], []
    for sec in (0, 1):                       # q section then k section
        for p in range(2):
            for e in range(2):
                h = h0 + 2 * p + e
                rows.append(w_qkv[sec * C + D * h: sec * C + D * h + D])
                brows.append(b_qkv[sec * C + D * h: sec * C + D * h + D])
    W_stack = np.concatenate(rows, 0)        # [512, 1024]
    w_qk = np.ascontiguousarray(W_stack.T.reshape(8, P, 512), BF)
    # only the q biases (first two 128-row groups); k bias is dropped
    # (it shifts every score in a row by the same amount -> softmax-inv).
    b_q = np.ascontiguousarray(
        np.concatenate(brows[:4], 0).reshape(2, P).T, np.float32)

    W_v4 = w_qkv[2 * C + D * h0: 2 * C + D * h0 + 256]
    w_v = np.ascontiguousarray(W_v4.T.reshape(8, P, 256), BF)

    w_p = np.ascontiguousarray(w_proj.T.reshape(8, P, 1024), BF)
    # effective proj bias per head: b_proj + w_proj @ tile(bv_h, 16)
    b_p = np.empty((4, P, 1024), np.float32)
    for hh in range(4):
        bv = b_qkv[2 * C + D * (h0 + hh): 2 * C + D * (h0 + hh) + D]
        eff = b_proj + w_proj @ np.tile(bv, 16)
        b_p[hh] = np.broadcast_to(eff, (P, 1024))
    b_p = np.ascontiguousarray(b_p, BF)
    tri = (np.arange(P)[None, :] >= np.arange(P)[:, None]).astype(BF)
    tri2 = np.ascontiguousarray(
        np.broadcast_to(tri[:, None, :], (P, 2, P)), BF)
    return {"xT": xT, "w_qk": w_qk, "w_v": w_v, "b_q": b_q,
            "w_p": w_p, "b_p": b_p, "tri2": tri2}


def _is_causal(mask: np.ndarray) -> bool:
    if mask.shape != (B, N, N):
        return False
    tril = np.tril(np.ones((N, N), bool))
    return bool(all(np.array_equal(mask[i], tril) for i in range(mask.shape[0])))


def _numpy_fallback(x, attention_mask, w_qkv, b_qkv, w_proj, b_proj):
    b, n, c = x.shape
    qkv = x @ w_qkv.T + b_qkv
    qkv = qkv.reshape(b, n, 3, H, D).transpose(2, 0, 3, 1, 4)
    q, k, v = qkv[0], qkv[1], qkv[2]
    dots = np.einsum("bhid,bhjd->bhij", q, k) * SCALE
    mask_value = -np.finfo(dots.dtype).max
    dots = np.where(attention_mask[:, None, :, :], dots, mask_value)
    dots = dots - dots.max(axis=-1, keepdims=True)
    e = np.exp(dots)
    attn = e / e.sum(axis=-1, keepdims=True)
    out = np.einsum("bhij,bhjd->bhid", attn, v)
    out = out.reshape(b, n, c)
    return (out @ w_proj.T + b_proj).astype(np.float32)


def kernel(**inputs) -> np.ndarray:
    x = np.asarray(inputs["x"], np.float32)
    mask = np.asarray(inputs["attention_mask"])
    w_qkv = np.asarray(inputs["w_qkv"], np.float32)
    b_qkv = np.asarray(inputs["b_qkv"], np.float32)
    w_proj = np.asarray(inputs["w_proj"], np.float32)
    b_proj = np.asarray(inputs["b_proj"], np.float32)

    if not _is_causal(mask):
        return _numpy_fallback(x, mask, w_qkv, b_qkv, w_proj, b_proj)

    nc = build_bass()
    in_maps = [_core_inputs(c, x, w_qkv, b_qkv, w_proj, b_proj)
               for c in range(8)]
    res = run_bass_kernel_spmd(nc, in_maps, core_ids=list(range(8)))
    out = np.empty((B, N, C), np.float32)
    for c in range(8):
        b = c // 4
        h0 = 4 * (c % 4)
        out[b, P * h0:P * h0 + 512, :] = res.results[c]["out"]
    return out


# revision 23
# speedup vs baseline: 2.0811x; 1.2762x over previous
"""Trainium2 Bass kernel for nn_MaskedAttention (B=2, N=2048, C=1024, H=16).

Sharding: batch x head-group over 8 cores (core c -> batch c//4, heads
4*(c%4)..4*(c%4)+3).  The reference's "faithful" head-scrambled reshape
means each head's output occupies a contiguous 128-row block of the
pre-projection matrix, so the output projection is row-parallel across
heads and needs no cross-core reduction.

Pipeline highlights (vs the straightforward version):
  - all matmul operands bf16 (halves input DMA, enables fast weight load);
    fp32 only in PSUM accumulators and the softmax denominator path.
  - k-bias dropped (softmax-invariant: it shifts each score row by a
    per-row constant); v-bias folded into an effective projection bias on
    the host (bp_eff = b_proj + w_proj @ tile(bv_h)); only q keeps its bias.
  - scores computed transposed sT[j,i] per 128x512 tile for both heads of
    a pair at once (row groups 0/64 -> concurrent on HW); exp reads the
    two heads' tiles as one [128,1024] PSUM-spanning activation.
  - AV uses the augmented-[V|1] stationary trick: row 64 of the PSUM
    output is the softmax denominator for free.
  - reciprocal on DVE, partition-broadcast on GPSIMD (Pool), diagonal-tile
    memsets on Pool; normalization multiply writes directly in the
    head-scrambled projection staging layout (no separate copies).
  - QKV / V / projection chains are interleaved into the attention stream
    as PE "filler" work so the PE never idles while ACT grinds exp.
"""

import numpy as np

import concourse.bass as bass
import concourse.mybir as mybir
from concourse import tile
from concourse.bass_utils import run_bass_kernel_spmd

B, N, C, H = 2, 2048, 1024, 16
D = C // H                 # 64
SCALE = D ** -0.5
EBIAS = -20.0
P = 128
NB = N // 512              # 4 n blocks
NJT = N // P               # 16 j tiles
F32 = mybir.dt.float32
BF16 = mybir.dt.bfloat16
AF = mybir.ActivationFunctionType

# Col-pack the two heads of a pair into one AV matmul round (tile_position
# (0,0)/(0,64)) with 4-way-concurrent M=1 denominator matmuls.  Dead end:
# each PSUM accumulation chain needs exclusive ownership of its bank's
# zero region (start=True clears the bank), so the packed layout needs 9
# banks against the 8 available.  Kept for documentation.
AV_PACK = False


def _emit(nc: bass.Bass, d: dict, repeats: int = 1):
    from contextlib import ExitStack

    with tile.TileContext(nc) as tc, ExitStack() as ctx:
        const = ctx.enter_context(tc.tile_pool(name="const", bufs=1))
        wqk = const.tile([P, 8, 512], BF16)
        wv = const.tile([P, 8, 256], BF16)
        bq = const.tile([P, 2], F32)
        tri2 = const.tile([P, 2, P], BF16)
        ebias = const.tile([P, 1], F32)
        qk = const.tile([P, 4, N], BF16)         # [p, {qq0,qq1,kk0,kk1}, n]
        xT = const.tile([P, 8, N], BF16)
        vaug = const.tile([P, NJT, 4 * 65], BF16)
        wp = const.tile([P, 8, 1024], BF16)
        bp = const.tile([P, 4, 1024], BF16)      # per-head effective proj bias

        nc.vector.memset(ebias[:], EBIAS)
        nc.gpsimd.memset(vaug[:], 1.0)
        ones1 = const.tile([P, 1], BF16)
        nc.vector.memset(ones1[:], 1.0)
        # pre-warm the exp table set while the first DMAs are in flight
        warm = const.tile([P, 1], F32)
        nc.scalar.activation(warm[:], ebias[:], AF.Exp)

        for _rep in range(repeats):
            with tc.tile_pool(name="sps", bufs=2, space="PSUM") as sps, \
                 tc.tile_pool(name="avps", bufs=1, space="PSUM") as avps, \
                 tc.tile_pool(name="fps", bufs=2, space="PSUM") as fps, \
                 tc.tile_pool(name="att", bufs=2) as att, \
                 tc.tile_pool(name="stg", bufs=2) as stg, \
                 tc.tile_pool(name="post", bufs=3) as post:
                # ---------- input DMAs ----------
                nc.sync.dma_start(
                    wqk[:, 0:2, :], d["w_qk"][0:2].rearrange("c p w -> p c w"))
                nc.sync.dma_start(
                    xT[:, 0:2, 0:512],
                    d["xT"][0, 0:2].rearrange("c p w -> p c w"))
                nc.sync.dma_start(
                    wqk[:, 2:8, :], d["w_qk"][2:8].rearrange("c p w -> p c w"))
                nc.sync.dma_start(
                    xT[:, 2:8, 0:512],
                    d["xT"][0, 2:8].rearrange("c p w -> p c w"))
                nc.sync.dma_start(bq[:], d["b_q"][:])
                nc.sync.dma_start(wv[:], d["w_v"].rearrange("c p w -> p c w"))
                for nb in range(1, NB):
                    nc.sync.dma_start(
                        xT[:, :, 512 * nb:512 * nb + 512],
                        d["xT"][nb].rearrange("c p w -> p c w"))
                nc.sync.dma_start(tri2[:], d["tri2"][:])
                nc.sync.dma_start(wp[:], d["w_p"].rearrange("c p w -> p c w"))
                nc.sync.dma_start(bp[:], d["b_p"].rearrange("h p w -> p h w"))

                # ---------- PE work-chunk emitters ----------
                def emit_qk_chain(mb, nb):
                    # q rows (mb 0/1) carry the bias; k rows (mb 2/3) don't
                    ps = fps.tile([P, 512], F32, tag="f")
                    for cc in range(8):
                        nc.tensor.matmul(
                            ps[:],
                            wqk[:, cc, P * mb:P * mb + P],
                            xT[:, cc, 512 * nb:512 * nb + 512],
                            start=(cc == 0), stop=(cc == 7),
                        )
                    dst = qk[:, mb, 512 * nb:512 * nb + 512]
                    if mb < 2:
                        nc.vector.tensor_scalar_add(dst, ps[:], bq[:, mb:mb + 1])
                    else:
                        nc.vector.tensor_copy(dst, ps[:])

                def emit_v_chain(jt):
                    ps = fps.tile([P, 512], F32, tag="f")
                    for cc in range(8):
                        nc.tensor.matmul(
                            ps[:, 0:256],
                            xT[:, cc, P * jt:P * jt + P],
                            wv[:, cc, :],
                            start=(cc == 0), stop=(cc == 7),
                        )
                    nc.vector.tensor_copy(
                        out=vaug[:, jt, :].rearrange(
                            "p (h x) -> p h x", x=65)[:, :, 0:64],
                        in_=ps[:, 0:256].rearrange("p (h x) -> p h x", x=64),
                    )

                def emit_proj_chunk(pair, hp, mb2, stage, osb):
                    h = 2 * pair + hp
                    ps = fps.tile([P, 512], F32, tag="f")
                    for k in range(8):
                        nc.tensor.matmul(
                            ps[:],
                            stage[:, k, :],
                            wp[:, k, 512 * mb2:512 * mb2 + 512],
                            start=(k == 0), stop=(k == 7),
                        )
                    nc.vector.tensor_add(
                        out=osb[:, 512 * mb2:512 * mb2 + 512], in0=ps[:],
                        in1=bp[:, h, 512 * mb2:512 * mb2 + 512])
                    nc.scalar.dma_start(
                        d["out"][P * h:P * h + P, 512 * mb2:512 * mb2 + 512],
                        osb[:, 512 * mb2:512 * mb2 + 512])

                # ---------- prologue: q/k for pair 0, v for jt 0-3 ----------
                for nb in range(NB):
                    emit_qk_chain(0, nb)
                    emit_qk_chain(2, nb)
                for jt in range(4):
                    emit_v_chain(jt)

                # ---------- attention with deadline-woven PE fillers ------
                proj_state = {}

                def run_filler(f):
                    if f[0] == "v":
                        emit_v_chain(f[1])
                    elif f[0] == "qk":
                        emit_qk_chain(f[1], f[2])
                    else:
                        _, pair, hp, mb2 = f
                        key = (pair, hp)
                        if key not in proj_state:
                            proj_state[key] = post.tile([P, 1024], F32, tag="osb",
                                                        name=f"osb{pair}{hp}")
                        emit_proj_chunk(pair, hp, mb2, stages[key],
                                        proj_state[key])

                def make_fillers(pair):
                    # ordered so force-draining to a deadline key is safe
                    fills = []
                    if pair == 0:
                        for jt in range(4, NJT):
                            fills.append(("v", jt))
                        for nb in range(NB):
                            fills.append(("qk", 1, nb))
                            fills.append(("qk", 3, nb))
                    else:
                        for hp in range(2):
                            for mb2 in range(2):
                                fills.append(("proj", 0, hp, mb2))
                    return fills

                stages = {}
                for pair in range(2):
                    fillers = make_fillers(pair)
                    fi = 0
                    nbatch = sum(4 * (m + 1) for m in range(NB))

                    def drain_until(fkey):
                        # emit fillers up to and including fkey (if pending)
                        nonlocal fi
                        if fkey not in fillers[fi:]:
                            return
                        stop = fillers.index(fkey, fi)
                        while fi <= stop:
                            run_filler(fillers[fi])
                            fi += 1
                    for hp in range(2):
                        stages[(pair, hp)] = stg.tile([P, 8, P], BF16, tag=f"stage{hp}", name=f"stage{pair}{hp}")
                    bi = 0
                    for m in range(NB):
                        njt = 4 * (m + 1)
                        expT = att.tile([P, NJT, 2, 512], BF16, tag="expT")
                        if AV_PACK:
                            av = avps.tile([P, 512], F32, tag="av0", name="av")
                            den = avps.tile([P, 512], F32, tag="av1",
                                            name="den")
                            pss = None
                        else:
                            pss = [avps.tile([65, 512], F32, tag=f"av{hp}",
                                             name=f"av{hp}")
                                   for hp in range(2)]

                        def emit_scores(jt):
                            s = sps.tile([P, 1024], F32, tag="s")
                            for hp in range(2):
                                lo = 64 * hp
                                nc.tensor.matmul(
                                    s[:, 512 * hp:512 * hp + 512],
                                    qk[lo:lo + 64, 2 + pair,
                                       P * jt:P * jt + P],
                                    qk[lo:lo + 64, pair,
                                       512 * m:512 * m + 512],
                                    start=True, stop=True,
                                )
                            t = jt - 4 * m
                            ex = expT[:, jt, :, :]
                            if t < 0:
                                nc.scalar.activation(
                                    ex.rearrange("p h w -> p (h w)"),
                                    s[:], AF.Exp, bias=ebias[:], scale=SCALE)
                            else:
                                if t > 0:
                                    nc.gpsimd.memset(ex[:, :, 0:P * t], 0.0)
                                nc.scalar.activation(
                                    ex[:, :, P * t:512],
                                    s[:].rearrange(
                                        "p (h w) -> p h w", h=2)[:, :, P * t:512],
                                    AF.Exp, bias=ebias[:], scale=SCALE)
                                nc.vector.tensor_mul(
                                    out=ex[:, :, P * t:P * t + P],
                                    in0=ex[:, :, P * t:P * t + P], in1=tri2[:])

                        def emit_av(jt):
                            if AV_PACK:
                                for hp in range(2):
                                    h = 2 * pair + hp
                                    nc.tensor.matmul(
                                        av[64 * hp:64 * hp + 64, :],
                                        vaug[:, jt, 65 * h:65 * h + 64],
                                        expT[:, jt, hp, :],
                                        start=(jt == 0), stop=(jt == njt - 1),
                                        tile_position=(0, 64 * hp),
                                    )
                                if jt % 2 == 1:
                                    # 4 concurrent M=1 denominator columns
                                    for jj in (jt - 1, jt):
                                        for hp in range(2):
                                            c = hp + 2 * (jj % 2)
                                            nc.tensor.matmul(
                                                den[32 * c:32 * c + 1, :],
                                                ones1[:],
                                                expT[:, jj, hp, :],
                                                start=(jj < 2),
                                                stop=(jj >= njt - 2),
                                                tile_position=(0, 32 * c),
                                            )
                            else:
                                for hp in range(2):
                                    h = 2 * pair + hp
                                    nc.tensor.matmul(
                                        pss[hp][:],
                                        vaug[:, jt, 65 * h:65 * h + 65],
                                        expT[:, jt, hp, :],
                                        start=(jt == 0), stop=(jt == njt - 1),
                                    )

                        # deadlines: q/k chains this m-block's scores need
                        if pair == 0 and m > 0:
                            drain_until(("qk", 2, m))
                        # software-pipelined: scores run two tiles ahead of AV
                        emit_scores(0)
                        if njt > 1:
                            emit_scores(1)
                        for jt in range(njt):
                            if jt + 2 < njt:
                                emit_scores(jt + 2)
                            if pair == 0:
                                drain_until(("v", jt))
                            emit_av(jt)
                            bi += 1
                            # weave PE filler chunks evenly across batches
                            if fi < len(fillers) and bi * len(fillers) >= \
                                    nbatch * (fi + 1):
                                run_filler(fillers[fi])
                                fi += 1
                        # normalization directly into the staged layout
                        for hp in range(2):
                            rec = post.tile([1, 512], F32, tag="rec")
                            if AV_PACK:
                                dsum = post.tile([1, 512], F32, tag="dsum")
                                nc.vector.tensor_add(
                                    out=dsum[:],
                                    in0=den[32 * hp:32 * hp + 1, :],
                                    in1=den[64 + 32 * hp:65 + 32 * hp, :])
                                nc.vector.reciprocal(rec[:], dsum[:])
                                avsrc = av[64 * hp:64 * hp + 64, :]
                            else:
                                ps_o = pss[hp]
                                nc.vector.reciprocal(rec[:], ps_o[64:65, :])
                                avsrc = ps_o[0:64, :]
                            bc = post.tile([64, 512], F32, tag="bc")
                            nc.gpsimd.dma_start(
                                bc[:],
                                rec[0:1, None, :].broadcast_to((1, 64, 512)))
                            stage = stages[(pair, hp)]
                            for e in range(2):
                                nc.vector.tensor_mul(
                                    out=stage[64 * e:64 * e + 64, :,
                                              32 * m:32 * m + 32],
                                    in0=avsrc.rearrange(
                                        "p (q k e) -> p k q e",
                                        k=8, e=2)[:, :, :, e],
                                    in1=bc[:, :].rearrange(
                                        "p (q k e) -> p k q e",
                                        k=8, e=2)[:, :, :, e],
                                )
                            if pair == 1 and m == NB - 1:
                                # tail: project this head as soon as staged
                                run_filler(("proj", 1, hp, 0))
                                run_filler(("proj", 1, hp, 1))
                    while fi < len(fillers):
                        run_filler(fillers[fi])
                        fi += 1


def _fix_bir_for_walrus(bir: bytes) -> bytes:
    """Split multi-semaphore-wait instructions for walrus builds that
    support only one sync-wait command per instruction: extra waits are
    hoisted onto same-engine NoOps inserted immediately before.  ISA-class
    (custom Pool) instructions get ALL waits hoisted."""
    import json as _json

    d = _json.loads(bir)
    uid = [0]
    for fn in d["functions"]:
        for blk in fn["blocks"]:
            out = []
            for inst in blk["instructions"]:
                si = inst.get("sync_info")
                waits = (si or {}).get("on_wait") or []
                keep = 0 if "isa_opcode" in inst else 1
                if len(waits) > keep:
                    hoist, rest = waits[:len(waits) - keep], waits[len(waits) - keep:]
                    for w in hoist:
                        uid[0] += 1
                        out.append({
                            "name": f"I-wsplit-{uid[0]}",
                            "opcode": "NoOp",
                            "engine": inst["engine"],
                            "ins": [],
                            "outs": [],
                            "sync_info": {"on_wait": [w], "on_update": []},
                        })
                    si["on_wait"] = rest
                out.append(inst)
            blk["instructions"] = out
    return _json.dumps(d).encode()


_NC_CACHE = None


def build_bass(repeats: int = 1) -> bass.Bass:
    global _NC_CACHE
    if repeats == 1 and _NC_CACHE is not None:
        return _NC_CACHE
    nc = bass.Bass("TRN2", target_bir_lowering=False, debug=False,
                   enable_asserts=False, num_devices=8)
    d = {
        "xT": nc.dram_tensor("xT", [NB, 8, P, 512], BF16, kind="ExternalInput").ap(),
        "w_qk": nc.dram_tensor("w_qk", [8, P, 512], BF16, kind="ExternalInput").ap(),
        "w_v": nc.dram_tensor("w_v", [8, P, 256], BF16, kind="ExternalInput").ap(),
        "b_q": nc.dram_tensor("b_q", [P, 2], F32, kind="ExternalInput").ap(),
        "w_p": nc.dram_tensor("w_p", [8, P, 1024], BF16, kind="ExternalInput").ap(),
        "b_p": nc.dram_tensor("b_p", [4, P, 1024], BF16, kind="ExternalInput").ap(),
        "tri2": nc.dram_tensor("tri2", [P, 2, P], BF16, kind="ExternalInput").ap(),
        "out": nc.dram_tensor("out", [512, 1024], F32, kind="ExternalOutput").ap(),
    }
    _emit(nc, d, repeats=repeats)
    _orig_to_json = nc.to_json_bytes
    nc.to_json_bytes = lambda: _fix_bir_for_walrus(_orig_to_json())
    if repeats == 1:
        _NC_CACHE = nc
    return nc


def _core_inputs(core: int, x, w_qkv, b_qkv, w_proj, b_proj) -> dict:
    import ml_dtypes

    BF = ml_dtypes.bfloat16
    b = core // 4
    h0 = 4 * (core % 4)
    xT = np.ascontiguousarray(
        x[b].T.reshape(8, P, NB, 512).transpose(2, 0, 1, 3), BF)

    rows, brows = [], []
    for sec in (0, 1):                       # q section then k section
        for p in range(2):
            for e in range(2):
                h = h0 + 2 * p + e
                rows.append(w_qkv[sec * C + D * h: sec * C + D * h + D])
                brows.append(b_qkv[sec * C + D * h: sec * C + D * h + D])
    W_stack = np.concatenate(rows, 0)        # [512, 1024]
    w_qk = np.ascontiguousarray(W_stack.T.reshape(8, P, 512), BF)
    # only the q biases (first two 128-row groups); k bias is dropped
    # (it shifts every score in a row by the same amount -> softmax-inv).
    b_q = np.ascontiguousarray(
        np.concatenate(brows[:4], 0).reshape(2, P).T, np.float32)

    W_v4 = w_qkv[2 * C + D * h0: 2 * C + D * h0 + 256]
    w_v = np.ascontiguousarray(W_v4.T.reshape(8, P, 256), BF)

    w_p = np.ascontiguousarray(w_proj.T.reshape(8, P, 1024), BF)
    # effective proj bias per head: b_proj + w_proj @ tile(bv_h, 16)
    b_p = np.empty((4, P, 1024), np.float32)
    for hh in range(4):
        bv = b_qkv[2 * C + D * (h0 + hh): 2 * C + D * (h0 + hh) + D]
        eff = b_proj + w_proj @ np.tile(bv, 16)
        b_p[hh] = np.broadcast_to(eff, (P, 1024))
    b_p = np.ascontiguousarray(b_p, BF)
    tri = (np.arange(P)[None, :] >= np.arange(P)[:, None]).astype(BF)
    tri2 = np.ascontiguousarray(
        np.broadcast_to(tri[:, None, :], (P, 2, P)), BF)
    return {"xT": xT, "w_qk": w_qk, "w_v": w_v, "b_q": b_q,
            "w_p": w_p, "b_p": b_p, "tri2": tri2}


def _is_causal(mask: np.ndarray) -> bool:
    if mask.shape != (B, N, N):
        return False
    tril = np.tril(np.ones((N, N), bool))
    return bool(all(np.array_equal(mask[i], tril) for i in range(mask.shape[0])))


def _numpy_fallback(x, attention_mask, w_qkv, b_qkv, w_proj, b_proj):
    b, n, c = x.shape
    qkv = x @ w_qkv.T + b_qkv
    qkv = qkv.reshape(b, n, 3, H, D).transpose(2, 0, 3, 1, 4)
    q, k, v = qkv[0], qkv[1], qkv[2]
    dots = np.einsum("bhid,bhjd->bhij", q, k) * SCALE
    mask_value = -np.finfo(dots.dtype).max
    dots = np.where(attention_mask[:, None, :, :], dots, mask_value)
    dots = dots - dots.max(axis=-1, keepdims=True)
    e = np.exp(dots)
    attn = e / e.sum(axis=-1, keepdims=True)
    out = np.einsum("bhij,bhjd->bhid", attn, v)
    out = out.reshape(b, n, c)
    return (out @ w_proj.T + b_proj).astype(np.float32)


def kernel(**inputs) -> np.ndarray:
    x = np.asarray(inputs["x"], np.float32)
    mask = np.asarray(inputs["attention_mask"])
    w_qkv = np.asarray(inputs["w_qkv"], np.float32)
    b_qkv = np.asarray(inputs["b_qkv"], np.float32)
    w_proj = np.asarray(inputs["w_proj"], np.float32)
    b_proj = np.asarray(inputs["b_proj"], np.float32)

    if not _is_causal(mask):
        return _numpy_fallback(x, mask, w_qkv, b_qkv, w_proj, b_proj)

    nc = build_bass()
    in_maps = [_core_inputs(c, x, w_qkv, b_qkv, w_proj, b_proj)
               for c in range(8)]
    res = run_bass_kernel_spmd(nc, in_maps, core_ids=list(range(8)))
    out = np.empty((B, N, C), np.float32)
    for c in range(8):
        b = c // 4
        h0 = 4 * (c % 4)
        out[b, P * h0:P * h0 + 512, :] = res.results[c]["out"]
    return out
